# revision 11
# baseline (speedup 1.0000x reference)
"""Causal self-attention (B=4, T=2048, D=1024, H=16) on 8 TRN2 NeuronCores.

Sharding: core c handles batch b = c//2 and head-group g = c%2 (8 heads each).
Each core computes, for its (b, g):
    qkv_loc = x[b] @ w_qkv[:, cols(g)]          (q|k|v local, 512 cols each)
    att     = causal_attention(q, k, v)          (8 heads, hd=64)
    y_part  = att @ w_out[rows(g), :]            ([2048, 1024] partial)
Host sums the two partial outputs per batch.

All matmuls run in bf16 with fp32 PSUM accumulation.  Softmax uses exp on
ScalarE with deferred normalization: rowsums come free from a ones-column
appended to V, the reciprocal is a single-pass Newton-Raphson approximation
read straight out of PSUM, and the result is broadcast across partitions
on the (otherwise idle) GpSimd engine.

Engine budget per core: PE streams ~240us of matmul columns; ScalarE's exp
stream is ~165us and cannot be reduced (ACT has no 2x mode for fp32 PSUM
reads), so emission order must keep independent PE work (projections for
the next query window, out-projection pieces) queued wherever exp is the
per-block limiter.

Phase structure (single emission stream; engines overlap via Tile deps):
  A  x -> xT (cast on GpSimd + PE transpose), V projection fused in,
     all Q/K weight pairs loaded, Q/K projection for query-window 0.
  C  query-window outer loop (it = 0..3), head inner: per head the causal
     scores / exp / attention-V chain, one-deep software-pipelined; the
     Q/K projection for window it+1 is interleaved one matmul-group per
     head; out-projection pieces for finished windows pop inside the
     jb2 loop, paced so most land in the exp-bound window 3.
Causal masking: key-blocks fully above the diagonal are skipped; the
scores matmul / attention-V matmul are narrowed to the live band and only
the 128x128 diagonal triangle gets a mask multiply.  Q is written by the
projection directly into per-parity zero-padded window buffers so the
packed-KT contraction (K=128 keeps the PE HAM clock gate warm) picks out
exactly one head.
"""

import numpy as np
from collections import deque
from contextlib import ExitStack

import concourse.bass as bass
import concourse.mybir as mybir
from concourse import bacc, tile
from concourse import bass_utils
from concourse.masks import make_identity

# Problem constants (hardcoded per contest contract)
B = 4
T = 2048
D = 1024
H = 16
HD = 64
H_LOC = 8               # heads per core
CLOC = H_LOC * HD       # 512 local head dims
P = 128
N_CORES = 8

F32 = mybir.dt.float32
BF16 = mybir.dt.bfloat16
MM_MODE = "bf16"


def _build_kernel_body(nc, tc, x_ap, wqkv_ap, wout_ap, out_ap):
    Exp = mybir.ActivationFunctionType.Exp
    mult = mybir.AluOpType.mult

    ctx = ExitStack()

    # ---------------- constants ----------------
    const = ctx.enter_context(tc.tile_pool(name="const", bufs=1))
    ident = const.tile([P, P], BF16)
    make_identity(nc, ident)
    # causal keep-mask for a 128x128 diagonal block: wm[k, q] = 1.0 iff q >= k
    wm = const.tile([P, P], BF16)
    nc.gpsimd.memset(wm, 1.0)
    nc.gpsimd.affine_select(
        out=wm,
        in_=wm,
        compare_op=mybir.AluOpType.is_ge,  # keep where f - p >= 0
        fill=0.0,
        base=0,
        channel_multiplier=-1,
        pattern=[[1, P]],
    )

    big = ctx.enter_context(tc.tile_pool(name="big", bufs=1))
    xT = big.tile([P, 8, T], BF16)      # [d%128, d//128, t]
    KT = big.tile([P, 4, T], BF16)      # head h -> rows (h%2)*64.., subtile h//2
    V_aug = big.tile([P, 16, H_LOC, HD + 1], BF16)  # [j%128, jb, h, dd|ones]
    AT = big.tile([P, 4, T], BF16)      # attention output, [dims, T] layout
    # padded Q: per parity, double-buffered by window.  Rows of the other
    # parity stay zero so the packed-KT scores contraction (K=128) selects
    # exactly one head.
    Qp0 = big.tile([P, 2, 4, 512], BF16)   # even heads; rows 64:128 zero
    Qp1 = big.tile([P, 2, 4, 512], BF16)   # odd heads; rows 0:64 zero
    nc.gpsimd.memset(V_aug[:, :, :, HD], 1.0)
    # Qp zero-halves are memset inside phase A (disjoint from the projection
    # writes, only read in phase C) so the in-order GpSimd stream serves the
    # startup x casts first

    xa = x_ap.rearrange("(tb p) d -> tb p d", p=P)          # [16, 128, 1024]
    wqk = wqkv_ap[:, 0:2 * CLOC].rearrange("(o p) c -> p o c", p=P)
    wv = wqkv_ap[:, 2 * CLOC:3 * CLOC].rearrange("(o p) c -> p o c", p=P)
    wo = wout_ap.rearrange("(o p) n -> p o n", p=P)         # [128, 4, 1024]
    oa = out_ap.rearrange("(tb p) d -> tb p d", p=P)

    wpre = ctx.enter_context(tc.tile_pool(name="wpre", bufs=1))
    wv_sb = wpre.tile([P, 8, CLOC], BF16)
    wo_sb = wpre.tile([P, 4, D], BF16)
    wo_st = wpre.tile([P, 4, D], F32)

    # ---- Q/K projection machinery: all 4 pairs stay resident ----
    ldw = ctx.enter_context(tc.tile_pool(name="ldw", bufs=1))
    psB = ctx.enter_context(tc.tile_pool(name="psB", bufs=1, space="PSUM"))

    def b_load_half(pair, ci):
        # ci = 0: Q columns (cb=pair), ci = 1: K columns (cb=pair+4)
        # unique wcb tags: all 8 weight tiles stay live through phase C
        cb = pair + 4 * ci
        wst = ldw.tile([P, 8, P], F32, tag=f"wst{cb % 3}")
        nc.sync.dma_start(wst, wqk[:, :, cb * P:(cb + 1) * P])
        wcb = ldw.tile([P, 8, P], BF16, tag=f"wcb{cb}")
        nc.vector.tensor_copy(wcb, wst)
        return wcb

    def bgroup(pair, wt, win, which):
        # project one (pair, Q|K, window) group; Q lands pre-padded in the
        # per-parity window buffers, K appends to the packed KT
        ps = psB.tile([P, 512], F32, tag="psb")
        for k in range(8):
            nc.tensor.matmul(
                ps,
                wt[:, k, :],
                xT[:, k, win * 512:(win + 1) * 512],
                start=(k == 0),
                stop=(k == 7),
            )
        if which == "k":
            nc.vector.tensor_copy(KT[:, pair, win * 512:(win + 1) * 512], ps)
        else:
            nc.vector.tensor_copy(Qp0[0:64, win % 2, pair, :], ps[0:64, :])
            nc.vector.tensor_copy(Qp1[64:128, win % 2, pair, :], ps[64:128, :])

    wpair = [[None, None] for _ in range(4)]

    # ---- phase A: x -> xT (cast+transpose), V projection fused,
    # Q/K window-0 projection for all pairs interleaved ----
    with tc.tile_pool(name="stage", bufs=1) as stage, \
         tc.tile_pool(name="lda", bufs=4) as lda, \
         tc.tile_pool(name="psA", bufs=4, space="PSUM") as psA, \
         tc.tile_pool(name="psV", bufs=2, space="PSUM") as psV:
        # warm the PE HAM clock gate (~3.4us of activity -> 2.4GHz) while
        # the first x tile and weights are still in flight
        psW = psV.tile([P, P], F32, tag="ps_v")
        for _ in range(33):
            nc.tensor.matmul(psW, ident, ident, start=True, stop=True)
        warm_sb = stage.tile([P, P], F32, tag="warm_sb")
        nc.vector.tensor_copy(warm_sb, psW)

        def load_x(tb):
            # two half-loads: finer DMA granularity keeps the PE fed at
            # startup (a >3.4us starve re-throttles the HAM clock gate)
            xcs = []
            for hf in (0, 1):
                xin = lda.tile([P, D // 2], F32, tag=f"xin{hf}")
                nc.sync.dma_start(xin, xa[tb][:, hf * 512:(hf + 1) * 512])
                xc = lda.tile([P, D // 2], BF16, tag=f"xc{hf}")
                nc.gpsimd.tensor_copy(xc, xin)  # cast on idle GpSimd
                xcs.append(xc)
            return xcs

        def vproj(tb):
            ps = psV.tile([P, CLOC], F32, tag="ps_v")
            for k in range(8):
                nc.tensor.matmul(
                    ps,
                    xT[:, k, tb * P:(tb + 1) * P],
                    wv_sb[:, k, :],
                    start=(k == 0),
                    stop=(k == 7),
                )
            nc.scalar.copy(
                V_aug[:, tb, :, 0:HD],
                ps.rearrange("p (h d) -> p h d", h=H_LOC),
            )

        def load_wv(j):
            wv_st = stage.tile([P, 2, CLOC], F32, tag=f"wv_st{j}")
            nc.sync.dma_start(wv_st, wv[:, 2 * j:2 * j + 2, :])
            nc.vector.tensor_copy(wv_sb[:, 2 * j:2 * j + 2, :], wv_st)

        # V-proj weights in 4 chunks so the first vproj isn't gated on the
        # whole 2MB load and the x stream keeps DMA priority.  The first two
        # x tiles are issued before any weight DMA so their completion never
        # queues behind a 512KB weight transfer.
        xc_cur = load_x(0)
        xc_next = load_x(1)
        load_wv(0)
        load_wv(1)

        for tb in range(T // P):
            xc_nn = load_x(tb + 2) if tb + 2 < T // P else None
            if tb == 0:
                load_wv(2)
                load_wv(3)
            # stagger the 8 Q/K weight half-loads across phase A; pair p's
            # halves land by tb=4p+2, its window-0 projection runs at 4p+3
            if 1 <= tb <= 8 and tb % 2 == 1:
                pr = tb // 2
                wpair[pr][0] = b_load_half(pr, 0)
            if 2 <= tb <= 9 and tb % 2 == 0:
                pr = (tb - 1) // 2
                wpair[pr][1] = b_load_half(pr, 1)
            # Qp zero-halves, split in four so the GpSimd cast stream for
            # the x tiles never backs up by more than ~1.7us
            if tb in (4, 6, 8, 10):
                which_qp = Qp0 if tb <= 6 else Qp1
                rows = slice(64, 128) if tb <= 6 else slice(0, 64)
                nc.gpsimd.memset(which_qp[rows, (tb // 2) % 2, :, :], 0.0)
            # one psum tile per transpose: a matmul with start=True clears
            # the whole destination bank, so slices of one bank can't be
            # written by separate transposes
            for db in range(8):
                pt = psA.tile([P, P], BF16, tag="pt")
                src_half = xc_cur[db // 4]
                nc.tensor.transpose(
                    pt, src_half[:, (db % 4) * P:(db % 4 + 1) * P], ident
                )
                nc.vector.tensor_copy(xT[:, db, tb * P:(tb + 1) * P], pt)
            if tb > 0:
                vproj(tb - 1)  # one-deep pipeline behind the transposes
            if tb % 4 == 3:
                pr = tb // 4
                bgroup(pr, wpair[pr][0], 0, "q")
                bgroup(pr, wpair[pr][1], 0, "k")
            xc_cur, xc_next = xc_next, xc_nn
        vproj(T // P - 1)

    # ---- phase C: causal attention, query-window outer ----
    # Scores matmuls contract over K=128 partitions (K<96 never warms the
    # PE HAM clock gate).  KT is packed (2 heads = 128 rows) as lhsT; the
    # moving Q operand is the per-parity window buffer with the other
    # head's 64 rows zeroed.
    attp = ctx.enter_context(tc.tile_pool(name="att", bufs=4))
    smp = ctx.enter_context(tc.tile_pool(name="sm", bufs=2))
    ypool = ctx.enter_context(tc.tile_pool(name="ypool", bufs=3))
    psS = ctx.enter_context(tc.tile_pool(name="psS", bufs=2, space="PSUM"))
    psO = ctx.enter_context(tc.tile_pool(name="psO", bufs=2, space="PSUM"))

    def norm(pend):
        # softmax normalization for a finished (head, window) block
        po, row0, sub, i0 = pend
        rs = smp.tile([1, 512], F32, tag="rs")
        # copy to partition 0 first: reciprocal_approx_fast (custom DVE op)
        # mishandles a nonzero input partition offset
        nc.vector.tensor_copy(rs, po[HD:HD + 1, :])
        rr = smp.tile([1, 512], F32, tag="rr")
        nc.vector.reciprocal_approx_fast(rr, rs)
        rm = smp.tile([1, 512], BF16, tag="rm")
        nc.vector.tensor_copy(rm, rr)
        # broadcast across partitions on the idle GpSimd engine (frees the
        # PE from the K=1 broadcast matmul); the multiply reads PSUM so it
        # must stay on DVE (GpSimd cannot access PSUM)
        rb = smp.tile([HD, 512], BF16, tag="rb")
        nc.gpsimd.partition_broadcast(rb, rm)
        nc.vector.tensor_tensor(
            AT[row0:row0 + 64, sub, i0:i0 + 512], po[0:HD, :], rb, mult
        )

    dctr = [0]

    def d_piece(tb, nt, on_scalar=False):
        # output projection for one (query block, half) piece; alternate
        # between two banks so copy-out never gates the next piece
        dctr[0] += 1
        py = psB.tile([P, 512], F32, tag="pb" if dctr[0] % 2 else "psb")
        for k in range(4):
            nc.tensor.matmul(
                py,
                AT[:, k, tb * P:(tb + 1) * P],
                wo_sb[:, k, nt * 512:(nt + 1) * 512],
                start=(k == 0),
                stop=(k == 3),
            )
        ysb = ypool.tile([P, 512], F32, tag="ysb")
        if on_scalar:
            nc.scalar.copy(ysb, py)
        else:
            nc.vector.tensor_copy(ysb, py)
        nc.sync.dma_start(oa[tb, :, nt * 512:(nt + 1) * 512], ysb)

    dq = deque()

    def flush(pending):
        # emit deferred softmax normalization; once head 7's window `it` is
        # normalized, the out-projection for that window becomes runnable
        h, po, row0, sub, i0 = pending
        norm((po, row0, sub, i0))
        if h == 7:
            for tb in range(4 * (i0 // 512), 4 * (i0 // 512) + 4):
                for nt in range(2):
                    dq.append((tb, nt))

    pending = None
    for it in range(4):
        i0 = it * 512
        njb = 4 * (it + 1)
        for sub in range(4):
            for parity in (0, 1):
                h = 2 * sub + parity
                row0 = parity * 64
                Qpw = (Qp0 if parity == 0 else Qp1)[:, it % 2, sub, :]
                if pending is not None:
                    flush(pending)
                    pending = None
                if it == 0 and h == 1:
                    # out-proj weights: late enough not to contend with the
                    # phase-A x stream, early enough for the first d_piece
                    nc.sync.dma_start(wo_st, wo)
                    nc.scalar.copy(wo_sb, wo_st)
                po = psO.tile([P, 512], F32, tag="po")

                def make_av(es_t, jb2_i):
                    def av():
                        for u in (0, 1):
                            jb = 2 * jb2_i + u
                            off = max(jb * P - i0, 0)
                            nc.tensor.matmul(
                                po[0:HD + 1, off:512],
                                V_aug[:, jb, h, :],
                                es_t[:, u * 512 + off:(u + 1) * 512],
                                start=(jb == 0),
                                stop=(jb == njb - 1),
                            )
                    return av

                prev_av = None
                for jb2 in range(njb // 2):
                    ps = psS.tile([P, 1024], F32, tag="ps_s")
                    es = attp.tile([P, 1024], BF16, tag="es")
                    for u in (0, 1):
                        jb = 2 * jb2 + u
                        off = max(jb * P - i0, 0)
                        nc.tensor.matmul(
                            ps[:, u * 512 + off:(u + 1) * 512],
                            KT[:, sub, jb * P:(jb + 1) * P],
                            Qpw[:, off:512],
                            start=True,
                            stop=True,
                        )
                    # exp trimmed to the live band (cols left of the u=0
                    # block's diagonal are never read downstream); stale
                    # cols inside [off0:1024] are also never read
                    off0 = max(2 * jb2 * P - i0, 0)
                    nc.scalar.activation(
                        es[:, off0:1024], ps[:, off0:1024], Exp, scale=0.125
                    )
                    for u in (0, 1):
                        jb = 2 * jb2 + u
                        off = jb * P - i0
                        if off >= 0:  # 128x128 diagonal triangle
                            nc.vector.tensor_tensor(
                                es[:, u * 512 + off:u * 512 + off + P],
                                es[:, u * 512 + off:u * 512 + off + P],
                                wm,
                                mult,
                            )
                    # attention-V for the PREVIOUS pair: one-deep software
                    # pipeline so the PE never idles behind exp+mask
                    if prev_av is not None:
                        prev_av()
                    prev_av = make_av(es, jb2)
                    # out-projection pieces pop paced so that most PE filler
                    # lands in the exp-bound later windows
                    want_pop = (
                        (it == 1 and parity == 1 and jb2 == 1)
                        or (it == 2 and jb2 == 2)
                        or (it == 3 and jb2 % 4 == 1)
                    )
                    if dq and want_pop:
                        d_piece(*dq.popleft(), on_scalar=(it <= 1))
                prev_av()
                pending = (h, po, row0, sub, i0)
                # project one Q/K group of the next window per head
                if it < 3:
                    which = "q" if parity == 0 else "k"
                    bgroup(sub, wpair[sub][parity], it + 1, which)
    flush(pending)
    k = 0
    while dq:
        d_piece(*dq.popleft(), on_scalar=(k % 2 == 1))
        k += 1

    ctx.close()


_CACHE = {}


def _get_nc(mode=None):
    if "nc" in _CACHE:
        return _CACHE["nc"]
    nc = bacc.Bacc(
        "TRN2",
        target_bir_lowering=False,
        debug=False,
        enable_asserts=False,
        num_devices=N_CORES,
    )
    x_d = nc.dram_tensor("x", [T, D], F32, kind="ExternalInput")
    wqkv_d = nc.dram_tensor("w_qkv", [D, 3 * CLOC], F32, kind="ExternalInput")
    wout_d = nc.dram_tensor("w_out", [CLOC, D], F32, kind="ExternalInput")
    out_d = nc.dram_tensor("out", [T, D], F32, kind="ExternalOutput")
    with tile.TileContext(nc) as tc:
        _build_kernel_body(
            nc, tc, x_d.ap(), wqkv_d.ap(), wout_d.ap(), out_d.ap()
        )
    nc.compile()
    _CACHE["nc"] = nc
    return nc


def _make_in_maps(x, w_qkv, w_out):
    x = np.ascontiguousarray(np.asarray(x, dtype=np.float32))
    w_qkv = np.ascontiguousarray(np.asarray(w_qkv, dtype=np.float32))
    w_out = np.ascontiguousarray(np.asarray(w_out, dtype=np.float32))
    in_maps = []
    for c in range(N_CORES):
        b, g = divmod(c, 2)
        c0 = g * CLOC
        wloc = np.concatenate(
            [
                w_qkv[:, c0:c0 + CLOC],
                w_qkv[:, D + c0:D + c0 + CLOC],
                w_qkv[:, 2 * D + c0:2 * D + c0 + CLOC],
            ],
            axis=1,
        )
        in_maps.append({
            "x": np.ascontiguousarray(x[b]),
            "w_qkv": np.ascontiguousarray(wloc),
            "w_out": np.ascontiguousarray(w_out[c0:c0 + CLOC]),
        })
    return in_maps


def run(x, w_qkv, w_out, trace=False, mode=None):
    nc = _get_nc(mode)
    in_maps = _make_in_maps(x, w_qkv, w_out)
    res = bass_utils.run_bass_kernel_spmd(
        nc, in_maps, core_ids=list(range(N_CORES)), trace=trace
    )
    y = np.empty((B, T, D), dtype=np.float32)
    for b in range(B):
        y[b] = res.results[2 * b]["out"] + res.results[2 * b + 1]["out"]
    return y, res


def kernel(x, w_qkv, w_out):
    y, _ = run(x, w_qkv, w_out, trace=False)
    return y


# revision 25
# speedup vs baseline: 1.0535x; 1.0535x over previous
"""Causal self-attention (B=4, T=2048, D=1024, H=16) on 8 TRN2 NeuronCores.

Sharding: core c handles batch b = c//2 and head-group g = c%2 (8 heads each).
Each core computes, for its (b, g):
    qkv_loc = x[b] @ w_qkv[:, cols(g)]          (q|k|v local, 512 cols each)
    att     = causal_attention(q, k, v)          (8 heads, hd=64)
    y_part  = att @ w_out[rows(g), :]            ([2048, 1024] partial)
Host sums the two partial outputs per batch.

All matmuls run in bf16 with fp32 PSUM accumulation.  Softmax uses exp on
ScalarE with deferred normalization: rowsums come free from a ones-column
appended to V, the reciprocal is a single-pass Newton-Raphson approximation
read straight out of PSUM, and the result is broadcast across partitions
on the (otherwise idle) GpSimd engine.

Engine budget per core: PE streams ~240us of matmul columns; ScalarE's exp
stream is ~165us and cannot be reduced (ACT has no 2x mode for fp32 PSUM
reads), so emission order must keep independent PE work (projections for
the next query window, out-projection pieces) queued wherever exp is the
per-block limiter.

Phase structure (single emission stream; engines overlap via Tile deps):
  A  x -> xT (cast on GpSimd + PE transpose), V projection fused in,
     all Q/K weight pairs loaded, Q/K projection for query-window 0.
  C  query-window outer loop (it = 0..3), head inner: per head the causal
     scores / exp / attention-V chain, one-deep software-pipelined; the
     Q/K projection for window it+1 is interleaved one matmul-group per
     head; out-projection pieces for finished windows pop inside the
     jb2 loop, paced so most land in the exp-bound window 3.
Causal masking: key-blocks fully above the diagonal are skipped; the
scores matmul / attention-V matmul are narrowed to the live band and only
the 128x128 diagonal triangle gets a mask multiply.  Q is written by the
projection directly into per-parity zero-padded window buffers so the
packed-KT contraction (K=128 keeps the PE HAM clock gate warm) picks out
exactly one head.
"""

import numpy as np
from collections import deque
from contextlib import ExitStack

import concourse.bass as bass
import concourse.mybir as mybir
from concourse import bacc, tile
from concourse import bass_utils
from concourse.masks import make_identity

# Problem constants (hardcoded per contest contract)
B = 4
T = 2048
D = 1024
H = 16
HD = 64
H_LOC = 8               # heads per core
CLOC = H_LOC * HD       # 512 local head dims
P = 128
N_CORES = 8

F32 = mybir.dt.float32
BF16 = mybir.dt.bfloat16
MM_MODE = "bf16"


def _build_kernel_body(nc, tc, x_ap, wqkv_ap, wout_ap, out_ap):
    Exp = mybir.ActivationFunctionType.Exp
    mult = mybir.AluOpType.mult

    ctx = ExitStack()

    # ---------------- constants ----------------
    const = ctx.enter_context(tc.tile_pool(name="const", bufs=1))
    ident = const.tile([P, P], BF16)
    make_identity(nc, ident)
    # causal keep-mask for a 128x128 diagonal block: wm[k, q] = 1.0 iff q >= k
    wm = const.tile([P, P], BF16)
    nc.gpsimd.memset(wm, 1.0)
    nc.gpsimd.affine_select(
        out=wm,
        in_=wm,
        compare_op=mybir.AluOpType.is_ge,  # keep where f - p >= 0
        fill=0.0,
        base=0,
        channel_multiplier=-1,
        pattern=[[1, P]],
    )

    big = ctx.enter_context(tc.tile_pool(name="big", bufs=1))
    xT = big.tile([P, 8, T], BF16)      # [d%128, d//128, t]
    KT = big.tile([P, 4, T], BF16)      # head h -> rows (h%2)*64.., subtile h//2
    V_aug = big.tile([P, 16, H_LOC, HD + 1], BF16)  # [j%128, jb, h, dd|ones]
    AT = big.tile([P, 4, T], BF16)      # attention output, [dims, T] layout
    # padded Q: per parity, double-buffered by window.  Rows of the other
    # parity stay zero so the packed-KT scores contraction (K=128) selects
    # exactly one head.
    Qp0 = big.tile([P, 2, 4, 512], BF16)   # even heads; rows 64:128 zero
    Qp1 = big.tile([P, 2, 4, 512], BF16)   # odd heads; rows 0:64 zero
    nc.gpsimd.memset(V_aug[:, :, :, HD], 1.0)
    # Qp zero-halves are memset inside phase A (disjoint from the projection
    # writes, only read in phase C) so the in-order GpSimd stream serves the
    # startup x casts first

    xa = x_ap.rearrange("(tb p) d -> tb p d", p=P)          # [16, 128, 1024]
    wqk = wqkv_ap[:, 0:2 * CLOC].rearrange("(o p) c -> p o c", p=P)
    wv = wqkv_ap[:, 2 * CLOC:3 * CLOC].rearrange("(o p) c -> p o c", p=P)
    wo = wout_ap.rearrange("(o p) n -> p o n", p=P)         # [128, 4, 1024]
    oa = out_ap.rearrange("(tb p) d -> tb p d", p=P)

    wpre = ctx.enter_context(tc.tile_pool(name="wpre", bufs=1))
    wv_sb = wpre.tile([P, 8, CLOC], BF16)
    wo_sb = wpre.tile([P, 4, D], BF16)
    wo_st = wpre.tile([P, 4, D], F32)

    # ---- Q/K projection machinery: all 4 pairs stay resident ----
    ldw = ctx.enter_context(tc.tile_pool(name="ldw", bufs=1))
    psB = ctx.enter_context(tc.tile_pool(name="psB", bufs=1, space="PSUM"))

    def b_load_half(pair, ci):
        # ci = 0: Q columns (cb=pair), ci = 1: K columns (cb=pair+4)
        # unique wcb tags: all 8 weight tiles stay live through phase C
        cb = pair + 4 * ci
        wst = ldw.tile([P, 8, P], F32, tag=f"wst{cb % 3}")
        nc.sync.dma_start(wst, wqk[:, :, cb * P:(cb + 1) * P])
        wcb = ldw.tile([P, 8, P], BF16, tag=f"wcb{cb}")
        nc.vector.tensor_copy(wcb, wst)
        return wcb

    def bgroup(pair, wt, win, which):
        # project one (pair, Q|K, window) group; Q lands pre-padded in the
        # per-parity window buffers, K appends to the packed KT
        ps = psB.tile([P, 512], F32, tag="psb")
        for k in range(8):
            nc.tensor.matmul(
                ps,
                wt[:, k, :],
                xT[:, k, win * 512:(win + 1) * 512],
                start=(k == 0),
                stop=(k == 7),
            )
        if which == "k":
            nc.vector.tensor_copy(KT[:, pair, win * 512:(win + 1) * 512], ps)
        else:
            nc.vector.tensor_copy(Qp0[0:64, win % 2, pair, :], ps[0:64, :])
            nc.vector.tensor_copy(Qp1[64:128, win % 2, pair, :], ps[64:128, :])

    wpair = [[None, None] for _ in range(4)]

    # ---- phase A: x -> xT (cast+transpose), V projection fused,
    # Q/K window-0 projection for all pairs interleaved ----
    with tc.tile_pool(name="stage", bufs=1) as stage, \
         tc.tile_pool(name="lda", bufs=4) as lda, \
         tc.tile_pool(name="psA", bufs=2, space="PSUM") as psA, \
         tc.tile_pool(name="psV", bufs=2, space="PSUM") as psV:
        # warm the PE HAM clock gate (~3.4us of activity -> 2.4GHz) while
        # the first x tile and weights are still in flight
        psW = psV.tile([P, P], F32, tag="ps_v")
        for _ in range(33):
            nc.tensor.matmul(psW, ident, ident, start=True, stop=True)
        warm_sb = stage.tile([P, P], F32, tag="warm_sb")
        nc.vector.tensor_copy(warm_sb, psW)

        def load_x(tb):
            # two half-loads: finer DMA granularity keeps the PE fed at
            # startup (a >3.4us starve re-throttles the HAM clock gate)
            xcs = []
            for hf in (0, 1):
                xin = lda.tile([P, D // 2], F32, tag=f"xin{hf}")
                nc.sync.dma_start(xin, xa[tb][:, hf * 512:(hf + 1) * 512])
                xc = lda.tile([P, D // 2], BF16, tag=f"xc{hf}")
                nc.scalar.copy(xc, xin)  # cast on ScalarE (GpSimd is 3x slower)
                xcs.append(xc)
            return xcs

        def vproj(tb):
            ps = psV.tile([P, CLOC], F32, tag="ps_v")
            for k in range(8):
                nc.tensor.matmul(
                    ps,
                    xT[:, k, tb * P:(tb + 1) * P],
                    wv_sb[:, k, :],
                    start=(k == 0),
                    stop=(k == 7),
                )
            nc.scalar.copy(
                V_aug[:, tb, :, 0:HD],
                ps.rearrange("p (h d) -> p h d", h=H_LOC),
            )

        def load_wv(j):
            wv_st = stage.tile([P, 2, CLOC], F32, tag=f"wv_st{j}")
            nc.sync.dma_start(wv_st, wv[:, 2 * j:2 * j + 2, :])
            nc.vector.tensor_copy(wv_sb[:, 2 * j:2 * j + 2, :], wv_st)

        # V-proj weights in 4 chunks so the first vproj isn't gated on the
        # whole 2MB load and the x stream keeps DMA priority.  The first two
        # x tiles are issued before any weight DMA so their completion never
        # queues behind a 512KB weight transfer.
        xc_cur = load_x(0)
        xc_next = load_x(1)
        load_wv(0)
        load_wv(1)

        for tb in range(T // P):
            xc_nn = load_x(tb + 2) if tb + 2 < T // P else None
            if tb == 0:
                load_wv(2)
                load_wv(3)
            # stagger the 8 Q/K weight half-loads across phase A; pair p's
            # halves land by tb=4p+2, its window-0 projection runs at 4p+3
            if 1 <= tb <= 8 and tb % 2 == 1:
                pr = tb // 2
                wpair[pr][0] = b_load_half(pr, 0)
            if 2 <= tb <= 9 and tb % 2 == 0:
                pr = (tb - 1) // 2
                wpair[pr][1] = b_load_half(pr, 1)
            # Qp zero-halves, split in four so the GpSimd cast stream for
            # the x tiles never backs up by more than ~1.7us
            if tb in (4, 6, 8, 10):
                which_qp = Qp0 if tb <= 6 else Qp1
                rows = slice(64, 128) if tb <= 6 else slice(0, 64)
                nc.gpsimd.memset(which_qp[rows, (tb // 2) % 2, :, :], 0.0)
            # four transposes share one PSUM bank: the first (start=True)
            # clears the whole bank, the rest accumulate onto zeros; one
            # wide DVE copy then drains all four (4x fewer PSUM reads)
            for hf in (0, 1):
                pt = psA.tile([P, 4 * P], BF16, tag=f"pt{hf}")
                for q in range(4):
                    nc.tensor.matmul(
                        pt[:, q * P:(q + 1) * P],
                        xc_cur[hf][:, q * P:(q + 1) * P],
                        ident,
                        is_transpose=True,
                        start=(q == 0),
                        stop=(q == 3),
                    )
                nc.vector.tensor_copy(
                    xT[:, 4 * hf:4 * hf + 4, tb * P:(tb + 1) * P],
                    pt.rearrange("p (f q) -> p f q", f=4),
                )
            if tb > 0:
                vproj(tb - 1)  # one-deep pipeline behind the transposes
            if tb % 4 == 3:
                pr = tb // 4
                bgroup(pr, wpair[pr][0], 0, "q")
                bgroup(pr, wpair[pr][1], 0, "k")
            xc_cur, xc_next = xc_next, xc_nn
        vproj(T // P - 1)

    # ---- phase C: causal attention, query-window outer ----
    # Scores matmuls contract over K=128 partitions (K<96 never warms the
    # PE HAM clock gate).  KT is packed (2 heads = 128 rows) as lhsT; the
    # moving Q operand is the per-parity window buffer with the other
    # head's 64 rows zeroed.
    attp = ctx.enter_context(tc.tile_pool(name="att", bufs=4))
    smp = ctx.enter_context(tc.tile_pool(name="sm", bufs=2))
    ypool = ctx.enter_context(tc.tile_pool(name="ypool", bufs=3))
    psS = ctx.enter_context(tc.tile_pool(name="psS", bufs=2, space="PSUM"))
    psO = ctx.enter_context(tc.tile_pool(name="psO", bufs=2, space="PSUM"))

    def norm(pend):
        # softmax normalization for a finished (head, window) block
        po, row0, sub, i0 = pend
        rs = smp.tile([1, 512], F32, tag="rs")
        # copy to partition 0 first: reciprocal_approx_fast (custom DVE op)
        # mishandles a nonzero input partition offset
        nc.vector.tensor_copy(rs, po[HD:HD + 1, :])
        rr = smp.tile([1, 512], F32, tag="rr")
        nc.vector.reciprocal_approx_fast(rr, rs)
        rm = smp.tile([1, 512], BF16, tag="rm")
        nc.vector.tensor_copy(rm, rr)
        # broadcast across partitions on the idle GpSimd engine (frees the
        # PE from the K=1 broadcast matmul); the multiply reads PSUM so it
        # must stay on DVE (GpSimd cannot access PSUM)
        rb = smp.tile([HD, 512], BF16, tag="rb")
        nc.gpsimd.partition_broadcast(rb, rm)
        nc.vector.tensor_tensor(
            AT[row0:row0 + 64, sub, i0:i0 + 512], po[0:HD, :], rb, mult
        )

    dctr = [0]

    def d_piece(tb, nt, on_scalar=False):
        # output projection for one (query block, half) piece; alternate
        # between two banks so copy-out never gates the next piece
        dctr[0] += 1
        py = psB.tile([P, 512], F32, tag="pb" if dctr[0] % 2 else "psb")
        for k in range(4):
            nc.tensor.matmul(
                py,
                AT[:, k, tb * P:(tb + 1) * P],
                wo_sb[:, k, nt * 512:(nt + 1) * 512],
                start=(k == 0),
                stop=(k == 3),
            )
        ysb = ypool.tile([P, 512], F32, tag="ysb")
        if on_scalar:
            nc.scalar.copy(ysb, py)
        else:
            nc.vector.tensor_copy(ysb, py)
        nc.sync.dma_start(oa[tb, :, nt * 512:(nt + 1) * 512], ysb)

    dq = deque()

    def flush(pending):
        # emit deferred softmax normalization; once head 7's window `it` is
        # normalized, the out-projection for that window becomes runnable
        h, po, row0, sub, i0 = pending
        norm((po, row0, sub, i0))
        if h == 7:
            for tb in range(4 * (i0 // 512), 4 * (i0 // 512) + 4):
                for nt in range(2):
                    dq.append((tb, nt))

    pending = None
    for it in range(4):
        i0 = it * 512
        njb = 4 * (it + 1)
        for sub in range(4):
            for parity in (0, 1):
                h = 2 * sub + parity
                row0 = parity * 64
                Qpw = (Qp0 if parity == 0 else Qp1)[:, it % 2, sub, :]
                if pending is not None:
                    flush(pending)
                    pending = None
                if it == 0 and h == 1:
                    # out-proj weights: late enough not to contend with the
                    # phase-A x stream, early enough for the first d_piece.
                    # Cast split on DVE so the in-order ScalarE exp stream
                    # never queues behind a 3.4us copy.
                    nc.sync.dma_start(wo_st, wo)
                    nc.vector.tensor_copy(wo_sb[:, 0:2, :], wo_st[:, 0:2, :])
                if it == 0 and h == 2:
                    nc.vector.tensor_copy(wo_sb[:, 2:4, :], wo_st[:, 2:4, :])
                po = psO.tile([P, 512], F32, tag="po")

                def make_av(es_t, jb2_i):
                    def av():
                        for u in (0, 1):
                            jb = 2 * jb2_i + u
                            off = max(jb * P - i0, 0)
                            nc.tensor.matmul(
                                po[0:HD + 1, off:512],
                                V_aug[:, jb, h, :],
                                es_t[:, u * 512 + off:(u + 1) * 512],
                                start=(jb == 0),
                                stop=(jb == njb - 1),
                            )
                    return av

                prev_av = None
                for jb2 in range(njb // 2):
                    ps = psS.tile([P, 1024], F32, tag="ps_s")
                    es = attp.tile([P, 1024], BF16, tag="es")
                    for u in (0, 1):
                        jb = 2 * jb2 + u
                        off = max(jb * P - i0, 0)
                        nc.tensor.matmul(
                            ps[:, u * 512 + off:(u + 1) * 512],
                            KT[:, sub, jb * P:(jb + 1) * P],
                            Qpw[:, off:512],
                            start=True,
                            stop=True,
                        )
                    # exp trimmed to the live band (cols left of the u=0
                    # block's diagonal are never read downstream); stale
                    # cols inside [off0:1024] are also never read
                    off0 = max(2 * jb2 * P - i0, 0)
                    nc.scalar.activation(
                        es[:, off0:1024], ps[:, off0:1024], Exp, scale=0.125
                    )
                    for u in (0, 1):
                        jb = 2 * jb2 + u
                        off = jb * P - i0
                        if off >= 0:  # 128x128 diagonal triangle
                            nc.vector.tensor_tensor(
                                es[:, u * 512 + off:u * 512 + off + P],
                                es[:, u * 512 + off:u * 512 + off + P],
                                wm,
                                mult,
                            )
                    # attention-V for the PREVIOUS pair: one-deep software
                    # pipeline so the PE never idles behind exp+mask
                    if prev_av is not None:
                        prev_av()
                    prev_av = make_av(es, jb2)
                    # out-projection pieces pop paced so that most PE filler
                    # lands in the exp-bound later windows
                    want_pop = (
                        (it == 1 and parity == 1 and jb2 == 1)
                        or (it == 2 and jb2 == 2)
                        or (it == 3 and jb2 % 4 == 1)
                    )
                    if dq and want_pop:
                        d_piece(*dq.popleft(), on_scalar=(it <= 1))
                prev_av()
                pending = (h, po, row0, sub, i0)
                # project one Q/K group of the next window per head
                if it < 3:
                    which = "q" if parity == 0 else "k"
                    bgroup(sub, wpair[sub][parity], it + 1, which)
    flush(pending)
    k = 0
    while dq:
        d_piece(*dq.popleft(), on_scalar=(k % 2 == 1))
        k += 1

    ctx.close()


_CACHE = {}


def _get_nc(mode=None):
    if "nc" in _CACHE:
        return _CACHE["nc"]
    nc = bacc.Bacc(
        "TRN2",
        target_bir_lowering=False,
        debug=False,
        enable_asserts=False,
        num_devices=N_CORES,
    )
    x_d = nc.dram_tensor("x", [T, D], F32, kind="ExternalInput")
    wqkv_d = nc.dram_tensor("w_qkv", [D, 3 * CLOC], F32, kind="ExternalInput")
    wout_d = nc.dram_tensor("w_out", [CLOC, D], F32, kind="ExternalInput")
    out_d = nc.dram_tensor("out", [T, D], F32, kind="ExternalOutput")
    with tile.TileContext(nc) as tc:
        _build_kernel_body(
            nc, tc, x_d.ap(), wqkv_d.ap(), wout_d.ap(), out_d.ap()
        )
    nc.compile()
    _CACHE["nc"] = nc
    return nc


def _make_in_maps(x, w_qkv, w_out):
    x = np.ascontiguousarray(np.asarray(x, dtype=np.float32))
    w_qkv = np.ascontiguousarray(np.asarray(w_qkv, dtype=np.float32))
    w_out = np.ascontiguousarray(np.asarray(w_out, dtype=np.float32))
    in_maps = []
    for c in range(N_CORES):
        b, g = divmod(c, 2)
        c0 = g * CLOC
        wloc = np.concatenate(
            [
                w_qkv[:, c0:c0 + CLOC],
                w_qkv[:, D + c0:D + c0 + CLOC],
                w_qkv[:, 2 * D + c0:2 * D + c0 + CLOC],
            ],
            axis=1,
        )
        in_maps.append({
            "x": np.ascontiguousarray(x[b]),
            "w_qkv": np.ascontiguousarray(wloc),
            "w_out": np.ascontiguousarray(w_out[c0:c0 + CLOC]),
        })
    return in_maps


def run(x, w_qkv, w_out, trace=False, mode=None):
    nc = _get_nc(mode)
    in_maps = _make_in_maps(x, w_qkv, w_out)
    res = bass_utils.run_bass_kernel_spmd(
        nc, in_maps, core_ids=list(range(N_CORES)), trace=trace
    )
    y = np.empty((B, T, D), dtype=np.float32)
    for b in range(B):
        y[b] = res.results[2 * b]["out"] + res.results[2 * b + 1]["out"]
    return y, res


def kernel(x, w_qkv, w_out):
    y, _ = run(x, w_qkv, w_out, trace=False)
    return y


# revision 36
# speedup vs baseline: 1.0923x; 1.0369x over previous
"""Causal self-attention (B=4, T=2048, D=1024, H=16) on 8 TRN2 NeuronCores.

Sharding: core c handles batch b = c//2 and head-group g = c%2 (8 heads each).
Each core computes, for its (b, g):
    qkv_loc = x[b] @ w_qkv[:, cols(g)]          (q|k|v local, 512 cols each)
    att     = causal_attention(q, k, v)          (8 heads, hd=64)
    y_part  = att @ w_out[rows(g), :]            ([2048, 1024] partial)
Host sums the two partial outputs per batch.

All matmuls run in bf16 with fp32 PSUM accumulation.  Softmax uses exp on
ScalarE with deferred normalization: rowsums come free from a ones-column
appended to V, the reciprocal is a single-pass Newton-Raphson approximation
read straight out of PSUM, and the result is broadcast across partitions
on the (otherwise idle) GpSimd engine.

Engine budget per core: PE streams ~240us of matmul columns; ScalarE's exp
stream is ~165us and cannot be reduced (ACT has no 2x mode for fp32 PSUM
reads), so emission order must keep independent PE work (projections for
the next query window, out-projection pieces) queued wherever exp is the
per-block limiter.

Phase structure (single emission stream; engines overlap via Tile deps):
  A  x -> xT (cast on GpSimd + PE transpose), V projection fused in,
     all Q/K weight pairs loaded, Q/K projection for query-window 0.
  C  query-window outer loop (it = 0..3), head inner: per head the causal
     scores / exp / attention-V chain, one-deep software-pipelined; the
     Q/K projection for window it+1 is interleaved one matmul-group per
     head; out-projection pieces for finished windows pop inside the
     jb2 loop, paced so most land in the exp-bound window 3.
Causal masking: key-blocks fully above the diagonal are skipped; the
scores matmul / attention-V matmul are narrowed to the live band and only
the 128x128 diagonal triangle gets a mask multiply.  Q is written by the
projection directly into per-parity zero-padded window buffers so the
packed-KT contraction (K=128 keeps the PE HAM clock gate warm) picks out
exactly one head.
"""

import numpy as np
from collections import deque
from contextlib import ExitStack

import concourse.bass as bass
import concourse.mybir as mybir
from concourse import bacc, tile
from concourse import bass_utils
from concourse.masks import make_identity

# Problem constants (hardcoded per contest contract)
B = 4
T = 2048
D = 1024
H = 16
HD = 64
H_LOC = 8               # heads per core
CLOC = H_LOC * HD       # 512 local head dims
P = 128
N_CORES = 8

F32 = mybir.dt.float32
BF16 = mybir.dt.bfloat16
MM_MODE = "bf16"


def _build_kernel_body(nc, tc, x_ap, wqkv_ap, wout_ap, out_ap):
    Exp = mybir.ActivationFunctionType.Exp
    mult = mybir.AluOpType.mult

    ctx = ExitStack()

    # ---------------- constants ----------------
    const = ctx.enter_context(tc.tile_pool(name="const", bufs=1))
    ident = const.tile([P, P], BF16)
    make_identity(nc, ident)
    # causal keep-mask for a 128x128 diagonal block: wm[k, q] = 1.0 iff q >= k
    wm = const.tile([P, P], BF16)
    nc.gpsimd.memset(wm, 1.0)
    nc.gpsimd.affine_select(
        out=wm,
        in_=wm,
        compare_op=mybir.AluOpType.is_ge,  # keep where f - p >= 0
        fill=0.0,
        base=0,
        channel_multiplier=-1,
        pattern=[[1, P]],
    )

    big = ctx.enter_context(tc.tile_pool(name="big", bufs=1))
    xT = big.tile([P, 8, T], BF16)      # [d%128, d//128, t]
    KT = big.tile([P, 4, T], BF16)      # head h -> rows (h%2)*64.., subtile h//2
    V_aug = big.tile([P, 16, H_LOC, HD + 1], BF16)  # [j%128, jb, h, dd|ones]
    AT = big.tile([P, 4, T], BF16)      # attention output, [dims, T] layout
    # padded Q: per parity, double-buffered by window.  Rows of the other
    # parity stay zero so the packed-KT scores contraction (K=128) selects
    # exactly one head.
    Qp0 = big.tile([P, 2, 4, 512], BF16)   # even heads; rows 64:128 zero
    Qp1 = big.tile([P, 2, 4, 512], BF16)   # odd heads; rows 0:64 zero
    nc.gpsimd.memset(V_aug[:, :, :, HD], 1.0)
    # warm the GpSimd extended-op library now: the first partition_broadcast
    # otherwise pays a ~4.4us Q7 ucode reload right at phase-C start, and
    # the in-order DVE stream cascades that stall into the PE.  After this
    # point GpSimd runs ONLY partition_broadcast (no library switches).
    pbw = const.tile([HD, P], BF16)
    nc.gpsimd.partition_broadcast(pbw, wm[0:1, :])
    # Qp zero-halves are memset on DVE inside phase A (disjoint from the
    # projection writes, only read in phase C)

    xa = x_ap.rearrange("(tb p) d -> tb p d", p=P)          # [16, 128, 1024]
    wqk = wqkv_ap[:, 0:2 * CLOC].rearrange("(o p) c -> p o c", p=P)
    wv = wqkv_ap[:, 2 * CLOC:3 * CLOC].rearrange("(o p) c -> p o c", p=P)
    wo = wout_ap.rearrange("(o p) n -> p o n", p=P)         # [128, 4, 1024]
    oa = out_ap.rearrange("(tb p) d -> tb p d", p=P)

    wpre = ctx.enter_context(tc.tile_pool(name="wpre", bufs=1))
    wv_sb = wpre.tile([P, 8, CLOC], BF16)
    wo_sb = wpre.tile([P, 4, D], BF16)
    wo_st = wpre.tile([P, 4, D], F32)

    # ---- Q/K projection machinery: all 4 pairs stay resident ----
    ldw = ctx.enter_context(tc.tile_pool(name="ldw", bufs=1))
    psB = ctx.enter_context(tc.tile_pool(name="psB", bufs=1, space="PSUM"))

    def b_load_half(pair, ci):
        # ci = 0: Q columns (cb=pair), ci = 1: K columns (cb=pair+4)
        # unique wcb tags: all 8 weight tiles stay live through phase C
        cb = pair + 4 * ci
        wst = ldw.tile([P, 8, P], F32, tag=f"wst{cb % 3}")
        nc.sync.dma_start(wst, wqk[:, :, cb * P:(cb + 1) * P])
        wcb = ldw.tile([P, 8, P], BF16, tag=f"wcb{cb}")
        nc.vector.tensor_copy(wcb, wst)
        return wcb

    def bgroup(pair, wt, win, which):
        # project one (pair, Q|K, window) group; Q lands pre-padded in the
        # per-parity window buffers, K appends to the packed KT
        ps = psB.tile([P, 512], F32, tag="psb")
        for k in range(8):
            nc.tensor.matmul(
                ps,
                wt[:, k, :],
                xT[:, k, win * 512:(win + 1) * 512],
                start=(k == 0),
                stop=(k == 7),
            )
        if which == "k":
            nc.vector.tensor_copy(KT[:, pair, win * 512:(win + 1) * 512], ps)
        else:
            nc.vector.tensor_copy(Qp0[0:64, win % 2, pair, :], ps[0:64, :])
            nc.vector.tensor_copy(Qp1[64:128, win % 2, pair, :], ps[64:128, :])

    wpair = [[None, None] for _ in range(4)]

    # ---- phase A: x -> xT (cast+transpose), V projection fused,
    # Q/K window-0 projection for all pairs interleaved ----
    with tc.tile_pool(name="stage", bufs=1) as stage, \
         tc.tile_pool(name="lda", bufs=4) as lda, \
         tc.tile_pool(name="psA", bufs=2, space="PSUM") as psA, \
         tc.tile_pool(name="psV", bufs=2, space="PSUM") as psV:
        # warm the PE HAM clock gate (~3.4us of activity -> 2.4GHz) while
        # the first x tile and weights are still in flight
        psW = psV.tile([P, P], F32, tag="ps_v")
        for _ in range(33):
            nc.tensor.matmul(psW, ident, ident, start=True, stop=True)
        warm_sb = stage.tile([P, P], F32, tag="warm_sb")
        nc.vector.tensor_copy(warm_sb, psW)

        def load_x(tb):
            # two half-loads: finer DMA granularity keeps the PE fed at
            # startup (a >3.4us starve re-throttles the HAM clock gate)
            xcs = []
            for hf in (0, 1):
                xin = lda.tile([P, D // 2], F32, tag=f"xin{hf}")
                nc.sync.dma_start(xin, xa[tb][:, hf * 512:(hf + 1) * 512])
                xc = lda.tile([P, D // 2], BF16, tag=f"xc{hf}")
                nc.scalar.copy(xc, xin)  # cast on ScalarE (GpSimd is 3x slower)
                xcs.append(xc)
            return xcs

        def vproj(tb):
            ps = psV.tile([P, CLOC], F32, tag="ps_v")
            for k in range(8):
                nc.tensor.matmul(
                    ps,
                    xT[:, k, tb * P:(tb + 1) * P],
                    wv_sb[:, k, :],
                    start=(k == 0),
                    stop=(k == 7),
                )
            nc.scalar.copy(
                V_aug[:, tb, :, 0:HD],
                ps.rearrange("p (h d) -> p h d", h=H_LOC),
            )

        def load_wv(j):
            # cast on ScalarE: the DVE is the phase-A bottleneck engine
            wv_st = stage.tile([P, 2, CLOC], F32, tag=f"wv_st{j}")
            nc.sync.dma_start(wv_st, wv[:, 2 * j:2 * j + 2, :])
            nc.scalar.copy(wv_sb[:, 2 * j:2 * j + 2, :], wv_st)

        # V-proj weights in 4 chunks so the first vproj isn't gated on the
        # whole 2MB load and the x stream keeps DMA priority.  The first two
        # x tiles are issued before any weight DMA so their completion never
        # queues behind a 512KB weight transfer.
        xc_cur = load_x(0)
        xc_next = load_x(1)
        load_wv(0)
        load_wv(1)

        for tb in range(T // P):
            xc_nn = load_x(tb + 2) if tb + 2 < T // P else None
            if tb == 0:
                load_wv(2)
                load_wv(3)
            # stagger the 8 Q/K weight half-loads across phase A; pair p's
            # halves land by tb=4p+2, its window-0 projection runs at 4p+3
            if 1 <= tb <= 8 and tb % 2 == 1:
                pr = tb // 2
                wpair[pr][0] = b_load_half(pr, 0)
            if 2 <= tb <= 9 and tb % 2 == 0:
                pr = (tb - 1) // 2
                wpair[pr][1] = b_load_half(pr, 1)
            # Qp zero-halves, split in four and kept off GpSimd (whose only
            # phase-C op family must stay partition_broadcast)
            if tb in (4, 6, 8, 10):
                which_qp = Qp0 if tb <= 6 else Qp1
                rows = slice(64, 128) if tb <= 6 else slice(0, 64)
                nc.vector.memset(which_qp[rows, (tb // 2) % 2, :, :], 0.0)
            # four transposes share one PSUM bank: the first (start=True)
            # clears the whole bank, the rest accumulate onto zeros; one
            # wide DVE copy then drains all four (4x fewer PSUM reads)
            for hf in (0, 1):
                pt = psA.tile([P, 4 * P], BF16, tag=f"pt{hf}")
                for q in range(4):
                    nc.tensor.matmul(
                        pt[:, q * P:(q + 1) * P],
                        xc_cur[hf][:, q * P:(q + 1) * P],
                        ident,
                        is_transpose=True,
                        start=(q == 0),
                        stop=(q == 3),
                    )
                nc.vector.tensor_copy(
                    xT[:, 4 * hf:4 * hf + 4, tb * P:(tb + 1) * P],
                    pt.rearrange("p (f q) -> p f q", f=4),
                )
            if tb > 1:
                vproj(tb - 2)  # two-deep: the first vproj waits on all wv
                # casts, one tb of extra slack hides that chain
            if tb % 4 == 3:
                pr = tb // 4
                bgroup(pr, wpair[pr][0], 0, "q")
                bgroup(pr, wpair[pr][1], 0, "k")
            xc_cur, xc_next = xc_next, xc_nn
        vproj(T // P - 2)
        vproj(T // P - 1)

    # ---- phase C: causal attention, query-window outer ----
    # Scores matmuls contract over K=128 partitions (K<96 never warms the
    # PE HAM clock gate).  KT is packed (2 heads = 128 rows) as lhsT; the
    # moving Q operand is the per-parity window buffer with the other
    # head's 64 rows zeroed.
    attp = ctx.enter_context(tc.tile_pool(name="att", bufs=4))
    smp = ctx.enter_context(tc.tile_pool(name="sm", bufs=2))
    ypool = ctx.enter_context(tc.tile_pool(name="ypool", bufs=3))
    psS = ctx.enter_context(tc.tile_pool(name="psS", bufs=2, space="PSUM"))
    psO = ctx.enter_context(tc.tile_pool(name="psO", bufs=2, space="PSUM"))

    def norm_pre(pend):
        # softmax normalization chain for a finished (head, window) block,
        # up to the partition broadcast; returns the AT-write closure so the
        # caller can emit it late (the broadcast's ~1us GpSimd latency must
        # not sit ahead of the next block's mask mults in DVE program order)
        po, row0, sub, i0 = pend
        rs = smp.tile([1, 512], F32, tag="rs")
        # copy to partition 0 first: reciprocal_approx_fast (custom DVE op)
        # mishandles a nonzero input partition offset
        nc.vector.tensor_copy(rs, po[HD:HD + 1, :])
        rr = smp.tile([1, 512], F32, tag="rr")
        nc.vector.reciprocal_approx_fast(rr, rs)
        rm = smp.tile([1, 512], BF16, tag="rm")
        nc.vector.tensor_copy(rm, rr)
        # broadcast across partitions on the idle GpSimd engine (frees the
        # PE from the K=1 broadcast matmul); the multiply reads PSUM so it
        # must stay on DVE (GpSimd cannot access PSUM)
        rb = smp.tile([HD, 512], BF16, tag="rb")
        nc.gpsimd.partition_broadcast(rb, rm)

        def at_write():
            nc.vector.tensor_tensor(
                AT[row0:row0 + 64, sub, i0:i0 + 512], po[0:HD, :], rb, mult
            )
        return at_write

    dctr = [0]

    def d_piece(tb, nt, on_scalar=False, dma_on_scalar=False):
        # output projection for one (query block, half) piece; alternate
        # between two banks so copy-out never gates the next piece
        dctr[0] += 1
        py = psB.tile([P, 512], F32, tag="pb" if dctr[0] % 2 else "psb")
        for k in range(4):
            nc.tensor.matmul(
                py,
                AT[:, k, tb * P:(tb + 1) * P],
                wo_sb[:, k, nt * 512:(nt + 1) * 512],
                start=(k == 0),
                stop=(k == 3),
            )
        ysb = ypool.tile([P, 512], F32, tag="ysb")
        if on_scalar:
            nc.scalar.copy(ysb, py)
        else:
            nc.vector.tensor_copy(ysb, py)
        # drain pieces issue their DMA from ACT: the Sync engine's
        # descriptor backlog otherwise delays the final stores by ~6us
        eng = nc.scalar if dma_on_scalar else nc.sync
        eng.dma_start(oa[tb, :, nt * 512:(nt + 1) * 512], ysb)

    dq = deque()

    def flush(pending):
        # emit deferred softmax normalization; once head 7's window `it` is
        # normalized, the out-projection for that window becomes runnable
        h, po, row0, sub, i0 = pending
        at = norm_pre((po, row0, sub, i0))
        if h == 7:
            for tb in range(4 * (i0 // 512), 4 * (i0 // 512) + 4):
                for nt in range(2):
                    dq.append((tb, nt))
        return at

    pending = None
    for it in range(4):
        i0 = it * 512
        njb = 4 * (it + 1)
        for sub in range(4):
            for parity in (0, 1):
                h = 2 * sub + parity
                row0 = parity * 64
                Qpw = (Qp0 if parity == 0 else Qp1)[:, it % 2, sub, :]
                at_post = None
                if pending is not None:
                    at_post = flush(pending)
                    pending = None
                if it == 0 and h == 1:
                    # out-proj weights: late enough not to contend with the
                    # phase-A x stream, early enough for the first d_piece.
                    # Cast split on DVE so the in-order ScalarE exp stream
                    # never queues behind a 3.4us copy.
                    nc.sync.dma_start(wo_st, wo)
                    nc.vector.tensor_copy(wo_sb[:, 0:2, :], wo_st[:, 0:2, :])
                if it == 0 and h == 2:
                    nc.vector.tensor_copy(wo_sb[:, 2:4, :], wo_st[:, 2:4, :])
                po = psO.tile([P, 512], F32, tag="po")

                def make_av(es_t, jb2_i):
                    def av():
                        for u in (0, 1):
                            jb = 2 * jb2_i + u
                            off = max(jb * P - i0, 0)
                            nc.tensor.matmul(
                                po[0:HD + 1, off:512],
                                V_aug[:, jb, h, :],
                                es_t[:, u * 512 + off:(u + 1) * 512],
                                start=(jb == 0),
                                stop=(jb == njb - 1),
                            )
                    return av

                prev_av = None
                for jb2 in range(njb // 2):
                    ps = psS.tile([P, 1024], F32, tag="ps_s")
                    es = attp.tile([P, 1024], BF16, tag="es")
                    for u in (0, 1):
                        jb = 2 * jb2 + u
                        off = max(jb * P - i0, 0)
                        nc.tensor.matmul(
                            ps[:, u * 512 + off:(u + 1) * 512],
                            KT[:, sub, jb * P:(jb + 1) * P],
                            Qpw[:, off:512],
                            start=True,
                            stop=True,
                        )
                    # exp trimmed to the live band (cols left of the u=0
                    # block's diagonal are never read downstream); stale
                    # cols inside [off0:1024] are also never read
                    off0 = max(2 * jb2 * P - i0, 0)
                    nc.scalar.activation(
                        es[:, off0:1024], ps[:, off0:1024], Exp, scale=0.125
                    )
                    for u in (0, 1):
                        jb = 2 * jb2 + u
                        off = jb * P - i0
                        if off >= 0:  # 128x128 diagonal triangle
                            nc.vector.tensor_tensor(
                                es[:, u * 512 + off:u * 512 + off + P],
                                es[:, u * 512 + off:u * 512 + off + P],
                                wm,
                                mult,
                            )
                    # attention-V for the PREVIOUS pair: one-deep software
                    # pipeline so the PE never idles behind exp+mask
                    if prev_av is not None:
                        prev_av()
                    prev_av = make_av(es, jb2)
                    # out-projection pieces pop paced so that most PE filler
                    # lands in the exp-bound later windows
                    want_pop = (
                        (it == 1 and parity == 1 and jb2 == 1)
                        or (it == 2 and jb2 == 2)
                        or (it == 3 and jb2 % 4 == 1)
                    )
                    if dq and want_pop:
                        d_piece(*dq.popleft(), on_scalar=(it <= 1))
                prev_av()
                # deferred AT write of the PREVIOUS block's normalization:
                # by now its GpSimd broadcast has long finished, so this
                # never stalls the DVE stream
                if at_post is not None:
                    at_post()
                pending = (h, po, row0, sub, i0)
                # project one Q/K group of the next window per head
                if it < 3:
                    which = "q" if parity == 0 else "k"
                    bgroup(sub, wpair[sub][parity], it + 1, which)
    flush(pending)()
    k = 0
    while dq:
        d_piece(*dq.popleft(), on_scalar=(k % 2 == 1), dma_on_scalar=True)
        k += 1

    ctx.close()


_CACHE = {}


def _get_nc(mode=None):
    if "nc" in _CACHE:
        return _CACHE["nc"]
    nc = bacc.Bacc(
        "TRN2",
        target_bir_lowering=False,
        debug=False,
        enable_asserts=False,
        num_devices=N_CORES,
    )
    x_d = nc.dram_tensor("x", [T, D], F32, kind="ExternalInput")
    wqkv_d = nc.dram_tensor("w_qkv", [D, 3 * CLOC], F32, kind="ExternalInput")
    wout_d = nc.dram_tensor("w_out", [CLOC, D], F32, kind="ExternalInput")
    out_d = nc.dram_tensor("out", [T, D], F32, kind="ExternalOutput")
    with tile.TileContext(nc) as tc:
        _build_kernel_body(
            nc, tc, x_d.ap(), wqkv_d.ap(), wout_d.ap(), out_d.ap()
        )
    nc.compile()
    _CACHE["nc"] = nc
    return nc


def _make_in_maps(x, w_qkv, w_out):
    x = np.ascontiguousarray(np.asarray(x, dtype=np.float32))
    w_qkv = np.ascontiguousarray(np.asarray(w_qkv, dtype=np.float32))
    w_out = np.ascontiguousarray(np.asarray(w_out, dtype=np.float32))
    in_maps = []
    for c in range(N_CORES):
        b, g = divmod(c, 2)
        c0 = g * CLOC
        wloc = np.concatenate(
            [
                w_qkv[:, c0:c0 + CLOC],
                w_qkv[:, D + c0:D + c0 + CLOC],
                w_qkv[:, 2 * D + c0:2 * D + c0 + CLOC],
            ],
            axis=1,
        )
        in_maps.append({
            "x": np.ascontiguousarray(x[b]),
            "w_qkv": np.ascontiguousarray(wloc),
            "w_out": np.ascontiguousarray(w_out[c0:c0 + CLOC]),
        })
    return in_maps


def run(x, w_qkv, w_out, trace=False, mode=None):
    nc = _get_nc(mode)
    in_maps = _make_in_maps(x, w_qkv, w_out)
    res = bass_utils.run_bass_kernel_spmd(
        nc, in_maps, core_ids=list(range(N_CORES)), trace=trace
    )
    y = np.empty((B, T, D), dtype=np.float32)
    for b in range(B):
        y[b] = res.results[2 * b]["out"] + res.results[2 * b + 1]["out"]
    return y, res


def kernel(x, w_qkv, w_out):
    y, _ = run(x, w_qkv, w_out, trace=False)
    return y
